# revision 16
# baseline (speedup 1.0000x reference)
"""Conv2DMod (StyleGAN-style modulated 3x3 conv) for 8 Trainium2 NeuronCores.

Math: out[b] = d[b,:] * conv2d(x[b], weight * (1+y[b])[None,:,None,None])
with d[b,o] = rsqrt(sum_{i,kh,kw} (weight[o,i,kh,kw]*(1+y[b,i]))^2 + eps).

Sharding: data-parallel over batch, one sample per core. Each core holds a
replica of the shared weight (in lhsT layout), modulates it by its own
(1+y[b]) on-device, computes the demodulation scale with two small PE
matmuls, and runs the conv as 2x8 PSUM tiles x 18 accumulating fp32r
matmuls (9 taps x 2 input-channel chunks, N=512 pixels).
"""

import numpy as np

import concourse.bacc as bacc
import concourse.mybir as mybir
import concourse.tile as tile
from concourse.bass_utils import run_bass_kernel_spmd

B, C, H, W = 8, 256, 64, 64
O = 256
HP, WP = H + 2, W + 2  # 66x66 zero-padded image
EPS = 1e-6
F32 = mybir.dt.float32
F32R = mybir.dt.float32r

_CACHE = {}


def _build():
    nc = bacc.Bacc("TRN2", target_bir_lowering=False)
    xpad_d = nc.dram_tensor("xpad", [128, 2 * HP * WP], F32R, kind="ExternalInput")
    wt_d = nc.dram_tensor("wt", [128, 18 * O], F32R, kind="ExternalInput")
    ym_d = nc.dram_tensor("ym", [128, 2], F32, kind="ExternalInput")
    out_d = nc.dram_tensor("out", [2, 128, H * W], F32, kind="ExternalOutput")

    with tile.TileContext(nc) as tc:
        with (
            tc.tile_pool(name="big", bufs=1) as big,
            tc.tile_pool(name="small", bufs=1) as small,
            tc.tile_pool(name="outp", bufs=2) as outp,
            tc.tile_pool(name="cpsum", bufs=6, space="PSUM") as cpsum,
            tc.tile_pool(name="wpsum", bufs=1, space="PSUM") as wpsum,
            tc.tile_pool(name="dpsum", bufs=1, space="PSUM") as dpsum,
        ):
            w_all = big.tile([128, 18 * O], F32R)
            x_all = big.tile([128, 2 * HP * WP], F32R)
            ym = small.tile([128, 2], F32)
            ones = small.tile([128, 1], F32)
            s2 = small.tile([128, 2 * O], F32)
            dtmp = small.tile([128, 2], F32)
            dsb = small.tile([128, 2], F32)
            eps_ap = small.tile([128, 1], F32)
            warm_in = small.tile([128, 512], mybir.dt.bfloat16)
            nc.vector.memset(eps_ap[:], EPS)
            nc.vector.memset(ones[:], 1.0)
            nc.vector.memset(warm_in[:], 0.0)

            # HAM warm-up: dummy matmuls on zeros while input DMA streams, so
            # the real conv stream starts at the 2.4GHz clock (K=8/8).
            warm_ps = wpsum.tile([128, 512], F32)
            for k in range(16):
                nc.tensor.matmul(
                    warm_ps[:], warm_in[:, 0:128], warm_in[:],
                    start=(k == 0), stop=(k == 15),
                )

            nc.sync.dma_start(ym[:], ym_d[:])
            # DMA order tuned so the first conv half-group (ic0 taps of
            # group 0) unblocks as early as possible: w is ic-major.
            def xdma(ic, r0, r1):
                sl = slice(ic * HP * WP + r0 * WP, ic * HP * WP + r1 * WP)
                nc.sync.dma_start(x_all[:, sl], xpad_d[:, sl])

            def wdma(j0, j1):
                sl = slice(j0 * O, j1 * O)
                nc.sync.dma_start(w_all[:, sl], wt_d[:, sl])

            # Dense-start order: the PE must see an uninterrupted stream or
            # the HAM clock gate re-throttles (sparse early starts measured
            # strictly worse). Weights first, then x in row bands.
            wdma(0, 9)      # all ic0 taps
            xdma(0, 0, 19)
            wdma(9, 18)     # all ic1 taps
            xdma(1, 0, 19)
            for r0, r1 in [(19, 44), (44, 66)]:
                for ic in range(2):
                    xdma(ic, r0, r1)

            # ym = 1 + y
            nc.vector.tensor_scalar_add(ym[:], ym[:], 1.0)

            # modulate weights in place (per block, gated only by its own DMA):
            # w[:, j-block] *= (1 + y_i)
            for ic in range(2):
                for kk in range(9):
                    j = ic * 9 + kk
                    blk = w_all[:, j * O : (j + 1) * O]
                    nc.vector.tensor_scalar_mul(blk, blk, ym[:, ic : ic + 1])

            # s2[:, ic*O+o] = sum_kk wmod[o, i, kk]^2 — squares on ACT (idle),
            # accumulation on DVE. Equivalent to sum w^2 * ym^2.
            sq_pool_tiles = []
            for ic in range(2):
                dst = s2[:, ic * O : (ic + 1) * O]
                for kk in range(9):
                    src = w_all[:, (ic * 9 + kk) * O : (ic * 9 + kk + 1) * O].bitcast(F32)
                    if kk == 0:
                        nc.scalar.square(dst, src)
                    else:
                        tmp = outp.tile([128, O], F32, tag="sqtmp", bufs=4)
                        nc.scalar.square(tmp[:], src)
                        nc.vector.tensor_add(dst, dst, tmp[:])

            x_view = x_all.rearrange("p (c r q) -> p c r q", c=2, r=HP)

            def conv_mms(oc, nk):
                ps = cpsum.tile([128, 512], F32, tag="cps")
                mmi = 0
                for ic in range(2):
                    for kk in range(9):
                        kh, kw = divmod(kk, 3)
                        j = ic * 9 + kk
                        lhsT = w_all[:, j * O + oc * 128 : j * O + oc * 128 + 128]
                        rhs = x_view[:, ic, nk * 8 + kh : nk * 8 + kh + 8, kw : kw + W]
                        nc.tensor.matmul(
                            ps[:], lhsT, rhs, start=(mmi == 0), stop=(mmi == 17)
                        )
                        mmi += 1
                return ps

            def drain(ps, oc, nk, osb):
                # NOTE: must be emitted AFTER demod(oc) — Tile deps are
                # emission-ordered; a forward read of dsb would not sync.
                nc.vector.tensor_scalar_mul(
                    osb[:, nk * 512 : (nk + 1) * 512], ps[:], dsb[:, oc : oc + 1]
                )
                nc.sync.dma_start(
                    out_d[oc, :, nk * 512 : (nk + 1) * 512],
                    osb[:, nk * 512 : (nk + 1) * 512],
                )

            def conv_group(oc, nk, osb):
                drain(conv_mms(oc, nk), oc, nk, osb)

            def demod(oc):
                # d[o] = 1/sqrt(sum_{i,kk} wmod[i,o,kk]^2 + eps), o on partitions
                dpre = dpsum.tile([128, 1], F32, tag="dpre")
                for ic in range(2):
                    nc.tensor.matmul(
                        dpre[:],
                        s2[:, ic * O + oc * 128 : ic * O + oc * 128 + 128],
                        ones[:],
                        start=(ic == 0),
                        stop=(ic == 1),
                    )
                nc.scalar.activation(
                    dtmp[:, oc : oc + 1],
                    dpre[:],
                    mybir.ActivationFunctionType.Sqrt,
                    bias=eps_ap[:, 0:1],
                )
                nc.vector.reciprocal(dsb[:, oc : oc + 1], dtmp[:, oc : oc + 1])

            osb0 = outp.tile([128, H * W], F32, tag="osb")
            pending = [conv_mms(0, nk) for nk in range(3)]
            demod(0)
            demod(1)
            for nk, ps in enumerate(pending):
                drain(ps, 0, nk, osb0)
            for nk in range(3, 8):
                conv_group(0, nk, osb0)
            osb1 = outp.tile([128, H * W], F32, tag="osb")
            for nk in range(8):
                conv_group(1, nk, osb1)
    nc.compile()
    return nc


def _get_nc():
    if "nc" not in _CACHE:
        _CACHE["nc"] = _build()
    return _CACHE["nc"]


def _prep_inputs(x, y, weight):
    x = np.ascontiguousarray(x, dtype=np.float32)
    y = np.ascontiguousarray(y, dtype=np.float32)
    weight = np.ascontiguousarray(weight, dtype=np.float32)
    # weight[o, i, kh, kw] -> wt[p, (ic*9+kk)*O + o] with i = ic*128+p
    wt = weight.transpose(2, 3, 1, 0).reshape(9, 2, 128, O).transpose(1, 0, 2, 3)
    wt = np.ascontiguousarray(wt.transpose(2, 0, 1, 3).reshape(128, 18 * O))
    in_maps = []
    for b in range(B):
        xp = np.pad(x[b], ((0, 0), (1, 1), (1, 1))).reshape(2, 128, HP * WP)
        xp = np.ascontiguousarray(xp.transpose(1, 0, 2).reshape(128, 2 * HP * WP))
        ymb = np.ascontiguousarray(y[b].reshape(2, 128).T)
        in_maps.append({"xpad": xp, "wt": wt, "ym": ymb})
    return in_maps


def kernel(x, y, weight, _run_kwargs=None):
    nc = _get_nc()
    in_maps = _prep_inputs(x, y, weight)
    kwargs = _run_kwargs or {}
    res = run_bass_kernel_spmd(nc, in_maps, core_ids=list(range(B)), **kwargs)
    out = np.empty((B, O, H, W), dtype=np.float32)
    for b in range(B):
        out[b] = res.results[b]["out"].reshape(O, H, W)
    if _run_kwargs is not None:
        _CACHE["last_result"] = res
    return out
